# revision 20
# baseline (speedup 1.0000x reference)
"""GAT (bipartite GATConv + mean-pool + 2 FC) on 8 Trainium2 NeuronCores.

Strategy: shard destination nodes across the 8 cores (each core owns N/8 dst
nodes and all edges pointing at them) so the segment softmax is fully local to
a core — no collectives needed.  The host stages each core's edge shard as a
dst-major packed copy of x_s rows (pure index manipulation); per core:

  Per dst tile of 128 nodes (one node per partition, degree-sorted so tiles
  have uniform run lengths L): K=128 block-diagonal bf16 matmuls compute
  [h_s | a_s] for two edge slots at a time, landing PSUM results directly in
  the [dst x slot] layout the segment ops need — no DRAM table, no indirect
  DMA.  The segment softmax (max-subtraction skipped: logits bounded, exp
  safe in fp32) and weighted message sum are DVE/ACT ops along each
  partition's run.  A host-prepared one-hot matmul pools relu(out)*W2 into
  per-batch partials; batch counts come from a host bincount.

Pad slots use a host-solved x-vector v_pad with v_pad @ (W*att_src) = -300
per head, so padded edges vanish under exp just like a pad table row.

Host work is limited to index manipulation (edge sorting / padding / packed
layout / one-hot build), weight folding, and the final unsharding reduction
of 8 x [128,1] partials.
"""

import numpy as np
import ml_dtypes

import concourse.bacc as bacc
import concourse.tile as tile
from concourse import mybir
from concourse.bass_utils import run_bass_kernel_spmd

F32 = mybir.dt.float32
BF16 = mybir.dt.bfloat16

N_CORES = 8
P = 128
HEADS = 3
CH = 12
HC = HEADS * CH          # 36
ROW = HC + 4             # matmul out per slot: 36 h | 3 a_s | 1 pad
PAD_A = -300.0           # pad-slot a_s value: exp(0.2*-300) = e^-60 ~ 0
NEG_SLOPE = 0.2
PSB = 512                # fp32 elems per PSUM bank (matmul blocks of 6*80)

_nc_cache = {}


def _build_nc(in_dim, n_dst_tiles, n_xt_dbl, PT_list):
    """Build the SPMD Bass program (identical for all cores)."""
    key = (in_dim, n_dst_tiles, n_xt_dbl, tuple(PT_list))
    if key in _nc_cache:
        return _nc_cache[key]

    PT_max = max(PT_list)
    Lg = ((PT_max + 5) // 6) * 12          # allocated slots per g tile
    xe_cols = sum(PT_list) * P

    nc = bacc.Bacc("TRN2", target_bir_lowering=False, debug=False)
    d_xe = nc.dram_tensor("xe", [2 * in_dim, xe_cols], BF16, kind="ExternalInput")
    d_xt = nc.dram_tensor("xt_p", [2 * in_dim, n_xt_dbl * P], BF16, kind="ExternalInput")
    d_oh = nc.dram_tensor("oh", [P, n_dst_tiles * P], BF16, kind="ExternalInput")
    d_wf2 = nc.dram_tensor("wf2", [2 * in_dim, 2 * ROW], BF16, kind="ExternalInput")
    d_wat2 = nc.dram_tensor("wat2", [2 * in_dim, 8], BF16, kind="ExternalInput")
    d_w2 = nc.dram_tensor("w2b", [P, HC], F32, kind="ExternalInput")
    d_bb = nc.dram_tensor("biasb", [P, HC], F32, kind="ExternalInput")
    d_q = nc.dram_tensor("q_out", [P, 1], F32, kind="ExternalOutput")

    with tile.TileContext(nc) as tc:
        with tc.tile_pool(name="const", bufs=1) as cpool, \
             tc.tile_pool(name="xload", bufs=3) as xpool, \
             tc.tile_pool(name="gat", bufs=4) as gpool, \
             tc.tile_pool(name="work", bufs=3) as wpool, \
             tc.tile_pool(name="psA", bufs=2, space="PSUM") as psA, \
             tc.tile_pool(name="psB", bufs=1, space="PSUM") as psB, \
             tc.tile_pool(name="psT", bufs=1, space="PSUM") as psT:

            # ---- constants into SBUF ----
            t_wf2 = cpool.tile([2 * in_dim, 2 * ROW], BF16)
            nc.sync.dma_start(t_wf2[:], d_wf2[:])
            t_wat2 = cpool.tile([2 * in_dim, 8], BF16)
            nc.sync.dma_start(t_wat2[:], d_wat2[:])
            t_w2 = cpool.tile([P, HC], F32)
            nc.sync.dma_start(t_w2[:], d_w2[:])
            t_bb = cpool.tile([P, HC], F32)
            nc.sync.dma_start(t_bb[:], d_bb[:])
            t_oh = cpool.tile([P, n_dst_tiles * P], BF16)
            nc.sync.dma_start(t_oh[:], d_oh[:])
            t_xt = cpool.tile([2 * in_dim, n_xt_dbl * P], BF16)
            nc.sync.dma_start(t_xt[:], d_xt[:])

            t_qacc = cpool.tile([P, 1], F32)
            nc.vector.memset(t_qacc[:], 0.0)

            # ---- a_t per dst double-tile -> resident SBUF ----
            # t_at[:, d*8+(t%2)*4+h] = a_t of node tile t=2d+(t%2), head h
            t_at = cpool.tile([P, n_xt_dbl * 8], F32)
            for d in range(n_xt_dbl):
                ps = psT.tile([P, 8], F32, space="PSUM", tag="psat")
                nc.tensor.matmul(
                    ps[:], lhsT=t_xt[:, d * P:(d + 1) * P], rhs=t_wat2[:],
                    start=True, stop=True)
                nc.scalar.copy(t_at[:, d * 8:(d + 1) * 8], ps[:])

            # ---- main loop over dst tiles ----
            xoff = 0
            for t in range(n_dst_tiles):
                PT = PT_list[t]
                L = 2 * PT
                xe = xpool.tile([2 * in_dim, PT_max * P], BF16, tag="xe")
                nc.sync.dma_start(xe[:, : PT * P], d_xe[:, xoff:xoff + PT * P])
                xoff += PT * P

                # h|a per edge slot via block-diag matmuls (2 slots / matmul,
                # rhs columns (c, half)-interleaved); chunks of 18 matmuls
                # fill one 3-bank PSUM tile -> one transposing copy into
                # channel-major g [p, c, l] so every phase-B op reads
                # contiguous along l
                g = gpool.tile([P, ROW * Lg], F32, tag="G")
                gT = g[:].rearrange("p (c l) -> p c l", l=Lg)
                for c0 in range(0, PT, 18):
                    nchunk = min(18, PT - c0)
                    nblk = (nchunk + 5) // 6
                    ps = psA.tile([P, 3 * PSB], F32, space="PSUM", tag="psa")
                    for j in range(nchunk):
                        nc.tensor.matmul(
                            ps[:, (j // 6) * PSB + (j % 6) * 2 * ROW:
                               (j // 6) * PSB + (j % 6 + 1) * 2 * ROW],
                            lhsT=xe[:, (c0 + j) * P:(c0 + j + 1) * P],
                            rhs=t_wf2[:], start=True, stop=True)
                    ps5 = ps[:].rearrange("p (jb x) -> p jb x", x=PSB)
                    for jb in range(nblk):
                        ps4 = ps5[:, jb, : 6 * 2 * ROW].rearrange(
                            "p (jj c half) -> p c jj half", c=ROW, half=2)
                        l0 = c0 * 2 + jb * 12
                        g4 = gT[:, :, l0:l0 + 12].rearrange(
                            "p c (jj half) -> p c jj half", half=2)
                        nc.scalar.copy(g4, ps4)

                atc = (t // 2) * 8 + (t % 2) * 4

                # logits T = a_s + a_t (h-major, contiguous), leaky fused
                tT = wpool.tile([P, 2 * PT_max * HEADS], F32, tag="T")
                T3 = tT[:, : L * HEADS].rearrange("p (h l) -> p h l", h=HEADS)
                at_b = t_at[:, atc:atc + HEADS].unsqueeze(2).to_broadcast(
                    (P, HEADS, L))
                nc.vector.tensor_tensor(
                    out=T3[:], in0=gT[:, HC:HC + HEADS, :L], in1=at_b,
                    op=mybir.AluOpType.add)
                tLR = wpool.tile([P, 2 * PT_max * HEADS], F32, tag="LR")
                nc.vector.scalar_tensor_tensor(
                    out=tLR[:, : L * HEADS], in0=tT[:, : L * HEADS],
                    scalar=NEG_SLOPE, in1=tT[:, : L * HEADS],
                    op0=mybir.AluOpType.mult, op1=mybir.AluOpType.max)
                tE = wpool.tile([P, 2 * PT_max * HEADS], F32, tag="E")
                nc.scalar.activation(
                    tE[:, : L * HEADS], tLR[:, : L * HEADS],
                    mybir.ActivationFunctionType.Exp)
                E3h = tE[:, : L * HEADS].rearrange("p (h l) -> p h l", h=HEADS)

                # denom + reciprocal
                t_den = wpool.tile([P, HEADS], F32, tag="den")
                nc.vector.tensor_reduce(
                    out=t_den[:], in_=E3h, axis=mybir.AxisListType.X,
                    op=mybir.AluOpType.add)
                t_rec = wpool.tile([P, HEADS], F32, tag="rec")
                nc.vector.reciprocal(t_rec[:], t_den[:])

                # weighted message sum U = sum_l e * h  (channel-major M)
                tM = wpool.tile([P, 2 * PT_max * HC], F32, tag="M")
                M3 = tM[:, : L * HC].rearrange("p (j l) -> p j l", l=L)
                e_b = E3h.unsqueeze(2).to_broadcast((P, HEADS, CH, L))
                nc.vector.tensor_tensor(
                    out=M3[:], in0=gT[:, 0:HC, :L], in1=e_b,
                    op=mybir.AluOpType.mult)
                tU = wpool.tile([P, HC], F32, tag="U")
                nc.vector.tensor_reduce(
                    out=tU[:], in_=M3, axis=mybir.AxisListType.X,
                    op=mybir.AluOpType.add)

                # out = relu(U / denom + bias)  (small V ops on gpsimd)
                tV = wpool.tile([P, HC], F32, tag="V")
                rec_b = t_rec[:].unsqueeze(2).to_broadcast((P, HEADS, CH))
                nc.gpsimd.tensor_tensor(
                    out=tV[:].rearrange("p (h c) -> p h c", c=CH),
                    in0=tU[:].rearrange("p (h c) -> p h c", c=CH),
                    in1=rec_b, op=mybir.AluOpType.mult)
                nc.gpsimd.tensor_tensor(
                    out=tV[:], in0=tV[:], in1=t_bb[:], op=mybir.AluOpType.add)
                nc.scalar.activation(tV[:], tV[:], mybir.ActivationFunctionType.Relu)

                # rv = sum(V * W2) fused into accumulator (bf16 for the
                # pooling matmul: lhsT/rhs dtypes must match)
                tR = wpool.tile([P, HC], F32, tag="R")
                tRV = wpool.tile([P, 1], BF16, tag="RV")
                nc.vector.scalar_tensor_tensor(
                    out=tR[:], in0=tV[:], scalar=1.0, in1=t_w2[:],
                    op0=mybir.AluOpType.mult, op1=mybir.AluOpType.mult,
                    accum_out=tRV[:])

                # pool into batches: q += onehot(bid)^T @ rv  (host one-hot)
                ps_q = psB.tile([P, 1], F32, space="PSUM", tag="q")
                nc.tensor.matmul(
                    ps_q[:], lhsT=t_oh[:, t * P:(t + 1) * P], rhs=tRV[:],
                    start=True, stop=True)
                nc.vector.tensor_tensor(
                    out=t_qacc[:], in0=t_qacc[:], in1=ps_q[:],
                    op=mybir.AluOpType.add)

            nc.sync.dma_start(d_q[:], t_qacc[:])
    nc.finalize()
    _nc_cache[key] = nc
    return nc


def _pack_dbl(x, n_dbl, in_dim):
    """Pack [n_dbl*256, in_dim] node-major features into the K=128
    block-diagonal lhsT layout [2*in_dim, n_dbl*128] (bf16)."""
    a = x.reshape(n_dbl, 2, P, in_dim)
    return np.ascontiguousarray(
        a.transpose(1, 3, 0, 2).reshape(2 * in_dim, n_dbl * P)
    ).astype(ml_dtypes.bfloat16)


def kernel(**inputs):
    x_s = np.asarray(inputs["x_s"], np.float32)
    x_t = np.asarray(inputs["x_t"], np.float32)
    edge_index = np.asarray(inputs["edge_index"])
    x_s_batch = np.asarray(inputs["x_s_batch"]).astype(np.int64)
    W = np.asarray(inputs["W"], np.float32)
    att_src = np.asarray(inputs["att_src"], np.float32)
    att_dst = np.asarray(inputs["att_dst"], np.float32)
    bias = np.asarray(inputs["bias"], np.float32)
    fc1_w = np.asarray(inputs["fc1_w"], np.float32)
    fc1_b = np.asarray(inputs["fc1_b"], np.float32)
    fc3_w = np.asarray(inputs["fc3_w"], np.float32)
    fc3_b = np.asarray(inputs["fc3_b"], np.float32)

    n_nodes, in_dim = x_s.shape
    src = edge_index[0].astype(np.int64)
    dst = edge_index[1].astype(np.int64)

    # ---- host: edge bucketing by destination (layout prep only) ----
    deg = np.bincount(dst, minlength=n_nodes)
    order = np.argsort(-deg, kind="stable")      # nodes by degree desc
    nodes_per_core = (n_nodes + N_CORES - 1) // N_CORES
    n_dst_tiles = (nodes_per_core + P - 1) // P
    L_list = []
    for t in range(n_dst_tiles):
        r0 = t * P * N_CORES
        Lt = max(1, int(deg[order[min(r0, n_nodes - 1)]]))
        L_list.append(Lt + (Lt & 1))             # force even (2 slots/matmul)
    PT_list = [Lt // 2 for Lt in L_list]
    n_xt_dbl = (n_dst_tiles + 1) // 2

    # edges sorted by dst -> per-node contiguous src runs
    e_order = np.argsort(dst, kind="stable")
    src_sorted = src[e_order].astype(np.int64)
    starts = np.searchsorted(dst[e_order], np.arange(n_nodes))

    # fold weights (host weight prep)
    wa_t = np.einsum("khc,hc->kh", W.reshape(in_dim, HEADS, CH), att_dst).astype(np.float32)
    wa_s = np.einsum("khc,hc->kh", W.reshape(in_dim, HEADS, CH), att_src).astype(np.float32)
    wfold = np.zeros((in_dim, ROW), np.float32)
    wfold[:, :HC] = W
    wfold[:, HC:HC + HEADS] = wa_s
    wf2 = np.zeros((2 * in_dim, 2 * ROW), np.float32)
    wf2[:in_dim, :ROW] = wfold
    wf2[in_dim:, ROW:] = wfold
    # interleave output columns to (c, half) so PSUM->g copies land
    # channel-major: new col c*2+half = old col half*ROW+c
    wf2 = np.ascontiguousarray(
        wf2.reshape(2 * in_dim, 2, ROW).transpose(0, 2, 1).reshape(
            2 * in_dim, 2 * ROW)).astype(ml_dtypes.bfloat16)
    wat4 = np.zeros((in_dim, 4), np.float32)
    wat4[:, :HEADS] = wa_t
    wat2 = np.zeros((2 * in_dim, 8), np.float32)
    wat2[:in_dim, :4] = wat4
    wat2[in_dim:, 4:] = wat4
    wat2 = wat2.astype(ml_dtypes.bfloat16)
    w2 = (fc1_w @ fc3_w)[:, 0].astype(np.float32)      # [36]
    w2b = np.tile(w2[None, :], (P, 1))
    biasb = np.tile(bias[None, :], (P, 1))

    # pad-slot x vector: v_pad @ wa_s = PAD_A for every head
    v_pad = wa_s @ np.linalg.solve(
        wa_s.T @ wa_s, np.full((HEADS,), PAD_A, np.float64)).astype(np.float32)
    x_ext = np.vstack([x_s, v_pad[None, :]])           # row n_nodes = pad

    slot_ar = {}
    for Lt in set(L_list):
        slot_ar[Lt] = np.arange(Lt)[None, :]

    in_maps = []
    for c in range(N_CORES):
        node_ids = order[c::N_CORES]             # this core's dst nodes, deg-sorted
        ncnt = len(node_ids)
        pad_nodes = n_dst_tiles * P - ncnt
        nodes_pad = np.concatenate(
            [node_ids, np.zeros(pad_nodes, np.int64)]) if pad_nodes else node_ids
        valid_row = np.arange(n_dst_tiles * P) < ncnt

        oh = np.zeros((P, n_dst_tiles * P), np.float32)
        xe_blocks = []
        for t in range(n_dst_tiles):
            Lt = L_list[t]
            nt = nodes_pad[t * P:(t + 1) * P]
            vr = valid_row[t * P:(t + 1) * P]
            lens = np.where(vr, deg[nt], 0)
            mask = slot_ar[Lt] < lens[:, None]           # [P, Lt]
            idt = np.full((P, Lt), n_nodes, np.int64)
            gather_pos = (starts[nt][:, None] + slot_ar[Lt])[mask]
            idt[mask] = src_sorted[gather_pos]
            Et = x_ext[idt]                              # [P, Lt, in_dim]
            Et = Et.reshape(P, Lt // 2, 2, in_dim).transpose(2, 3, 1, 0)
            xe_blocks.append(np.ascontiguousarray(
                Et.reshape(2 * in_dim, (Lt // 2) * P)))
            rows = np.nonzero(vr)[0]
            oh[rows, t * P + x_s_batch[nt[rows]]] = 1.0
        xe = np.concatenate(xe_blocks, axis=1).astype(ml_dtypes.bfloat16)
        oh = oh.astype(ml_dtypes.bfloat16)

        valid = min(ncnt, n_dst_tiles * P)
        xt_pad = np.zeros((n_xt_dbl * 2 * P, in_dim), np.float32)
        xt_pad[:valid] = x_t[node_ids[:valid]]
        xt_p = _pack_dbl(xt_pad, n_xt_dbl, in_dim)
        in_maps.append({
            "xe": xe, "xt_p": xt_p, "oh": oh,
            "wf2": wf2, "wat2": wat2, "w2b": w2b, "biasb": biasb,
        })

    nc = _build_nc(in_dim, n_dst_tiles, n_xt_dbl, PT_list)
    res = run_bass_kernel_spmd(nc, in_maps, core_ids=list(range(N_CORES)))

    q = np.zeros((P,), np.float64)
    for c in range(N_CORES):
        q += res.results[c]["q_out"][:, 0]
    cnt = np.bincount(x_s_batch, minlength=P).astype(np.float64)
    out = q / np.maximum(cnt, 1.0)
    const = float(fc1_b @ fc3_w[:, 0] + fc3_b[0])
    return (out + const).astype(np.float32)


# revision 21
# speedup vs baseline: 1.3242x; 1.3242x over previous
"""GAT (bipartite GATConv + mean-pool + 2 FC) on 8 Trainium2 NeuronCores.

Strategy: shard destination nodes across the 8 cores (each core owns N/8 dst
nodes and all edges pointing at them) so the segment softmax is fully local to
a core — no collectives needed.  The host stages each core's edge shard as a
dst-major packed copy of x_s rows (pure index manipulation); per core:

  Per dst tile of 128 nodes (one node per partition, degree-sorted so tiles
  have uniform run lengths L): K=128 block-diagonal bf16 matmuls compute
  [h_s | a_s] for two edge slots at a time, landing PSUM results directly in
  the [dst x slot] layout the segment ops need — no DRAM table, no indirect
  DMA.  The segment softmax (max-subtraction skipped: logits bounded, exp
  safe in fp32) and weighted message sum are DVE/ACT ops along each
  partition's run.  A host-prepared one-hot matmul pools relu(out)*W2 into
  per-batch partials; batch counts come from a host bincount.

Pad slots use a host-solved x-vector v_pad with v_pad @ (W*att_src) = -300
per head, so padded edges vanish under exp just like a pad table row.

Host work is limited to index manipulation (edge sorting / padding / packed
layout / one-hot build), weight folding, and the final unsharding reduction
of 8 x [128,1] partials.
"""

import numpy as np
import ml_dtypes

import concourse.bacc as bacc
import concourse.tile as tile
from concourse import mybir
from concourse.bass_utils import run_bass_kernel_spmd

F32 = mybir.dt.float32
BF16 = mybir.dt.bfloat16

N_CORES = 8
P = 128
HEADS = 3
CH = 12
HC = HEADS * CH          # 36
ROW = HC + 4             # matmul out per slot: 36 h | 3 a_s | 1 pad
PAD_A = -300.0           # pad-slot a_s value: exp(0.2*-300) = e^-60 ~ 0
NEG_SLOPE = 0.2
PSB = 512                # fp32 elems per PSUM bank (matmul blocks of 6*80)

_nc_cache = {}


def _build_nc(in_dim, n_dst_tiles, n_xt_dbl, PT_list):
    """Build the SPMD Bass program (identical for all cores)."""
    key = (in_dim, n_dst_tiles, n_xt_dbl, tuple(PT_list))
    if key in _nc_cache:
        return _nc_cache[key]

    PT_max = max(PT_list)
    Lg = ((PT_max + 5) // 6) * 12          # allocated slots per g tile
    xe_cols = sum(PT_list) * P

    nc = bacc.Bacc("TRN2", target_bir_lowering=False, debug=False)
    d_xe = nc.dram_tensor("xe", [2 * in_dim, xe_cols], BF16, kind="ExternalInput")
    d_xt = nc.dram_tensor("xt_p", [2 * in_dim, n_xt_dbl * P], BF16, kind="ExternalInput")
    d_oh = nc.dram_tensor("oh", [P, n_dst_tiles * P], BF16, kind="ExternalInput")
    d_wf2 = nc.dram_tensor("wf2", [2 * in_dim, 2 * ROW], BF16, kind="ExternalInput")
    d_wat2 = nc.dram_tensor("wat2", [2 * in_dim, 8], BF16, kind="ExternalInput")
    d_w2 = nc.dram_tensor("w2b", [P, HC], F32, kind="ExternalInput")
    d_bb = nc.dram_tensor("biasb", [P, HC], F32, kind="ExternalInput")
    d_q = nc.dram_tensor("q_out", [P, 1], F32, kind="ExternalOutput")

    with tile.TileContext(nc) as tc:
        with tc.tile_pool(name="const", bufs=1) as cpool, \
             tc.tile_pool(name="xload", bufs=3) as xpool, \
             tc.tile_pool(name="gat", bufs=4) as gpool, \
             tc.tile_pool(name="work", bufs=3) as wpool, \
             tc.tile_pool(name="psA", bufs=2, space="PSUM") as psA, \
             tc.tile_pool(name="psB", bufs=2, space="PSUM") as psB, \
             tc.tile_pool(name="psT", bufs=1, space="PSUM") as psT:

            # ---- constants into SBUF ----
            t_wf2 = cpool.tile([2 * in_dim, 2 * ROW], BF16)
            nc.sync.dma_start(t_wf2[:], d_wf2[:])
            t_wat2 = cpool.tile([2 * in_dim, 8], BF16)
            nc.sync.dma_start(t_wat2[:], d_wat2[:])
            t_w2 = cpool.tile([P, HC], F32)
            nc.sync.dma_start(t_w2[:], d_w2[:])
            t_bb = cpool.tile([P, HC], F32)
            nc.sync.dma_start(t_bb[:], d_bb[:])
            t_oh = cpool.tile([P, n_dst_tiles * P], BF16)
            nc.sync.dma_start(t_oh[:], d_oh[:])
            t_xt = cpool.tile([2 * in_dim, n_xt_dbl * P], BF16)
            nc.sync.dma_start(t_xt[:], d_xt[:])

            t_qacc = cpool.tile([P, 1], F32)
            nc.vector.memset(t_qacc[:], 0.0)

            # ---- a_t per dst double-tile -> resident SBUF ----
            # t_at[:, d*8+(t%2)*4+h] = a_t of node tile t=2d+(t%2), head h
            t_at = cpool.tile([P, n_xt_dbl * 8], F32)
            for d in range(n_xt_dbl):
                ps = psT.tile([P, 8], F32, space="PSUM", tag="psat")
                nc.tensor.matmul(
                    ps[:], lhsT=t_xt[:, d * P:(d + 1) * P], rhs=t_wat2[:],
                    start=True, stop=True)
                nc.scalar.copy(t_at[:, d * 8:(d + 1) * 8], ps[:])

            # ---- main loop over dst tiles ----
            xoff = 0
            for t in range(n_dst_tiles):
                PT = PT_list[t]
                L = 2 * PT
                xe = xpool.tile([2 * in_dim, PT_max * P], BF16, tag="xe")
                nc.sync.dma_start(xe[:, : PT * P], d_xe[:, xoff:xoff + PT * P])
                xoff += PT * P

                # h|a per edge slot via block-diag matmuls (2 slots / matmul,
                # rhs columns (c, half)-interleaved); chunks of 18 matmuls
                # fill one 3-bank PSUM tile -> one transposing copy into
                # channel-major g [p, c, l] so every phase-B op reads
                # contiguous along l
                g = gpool.tile([P, ROW * Lg], F32, tag="G")
                gT = g[:].rearrange("p (c l) -> p c l", l=Lg)
                for c0 in range(0, PT, 12):
                    nchunk = min(12, PT - c0)
                    nblk = (nchunk + 5) // 6
                    ps = psA.tile([P, 2 * PSB], F32, space="PSUM", tag="psa")
                    for j in range(nchunk):
                        nc.tensor.matmul(
                            ps[:, (j // 6) * PSB + (j % 6) * 2 * ROW:
                               (j // 6) * PSB + (j % 6 + 1) * 2 * ROW],
                            lhsT=xe[:, (c0 + j) * P:(c0 + j + 1) * P],
                            rhs=t_wf2[:], start=True, stop=True)
                    ps5 = ps[:].rearrange("p (jb x) -> p jb x", x=PSB)
                    for jb in range(nblk):
                        ps4 = ps5[:, jb, : 6 * 2 * ROW].rearrange(
                            "p (jj c half) -> p c jj half", c=ROW, half=2)
                        l0 = c0 * 2 + jb * 12
                        g4 = gT[:, :, l0:l0 + 12].rearrange(
                            "p c (jj half) -> p c jj half", half=2)
                        nc.scalar.copy(g4, ps4)

                atc = (t // 2) * 8 + (t % 2) * 4

                # logits T = a_s + a_t (h-major, contiguous), leaky fused
                tT = wpool.tile([P, 2 * PT_max * HEADS], F32, tag="T")
                T3 = tT[:, : L * HEADS].rearrange("p (h l) -> p h l", h=HEADS)
                at_b = t_at[:, atc:atc + HEADS].unsqueeze(2).to_broadcast(
                    (P, HEADS, L))
                nc.vector.tensor_tensor(
                    out=T3[:], in0=gT[:, HC:HC + HEADS, :L], in1=at_b,
                    op=mybir.AluOpType.add)
                tLR = wpool.tile([P, 2 * PT_max * HEADS], F32, tag="LR")
                nc.vector.scalar_tensor_tensor(
                    out=tLR[:, : L * HEADS], in0=tT[:, : L * HEADS],
                    scalar=NEG_SLOPE, in1=tT[:, : L * HEADS],
                    op0=mybir.AluOpType.mult, op1=mybir.AluOpType.max)
                tE = wpool.tile([P, 2 * PT_max * HEADS], F32, tag="E")
                nc.scalar.activation(
                    tE[:, : L * HEADS], tLR[:, : L * HEADS],
                    mybir.ActivationFunctionType.Exp)
                E3h = tE[:, : L * HEADS].rearrange("p (h l) -> p h l", h=HEADS)

                # denom + reciprocal
                t_den = wpool.tile([P, HEADS], F32, tag="den")
                nc.vector.tensor_reduce(
                    out=t_den[:], in_=E3h, axis=mybir.AxisListType.X,
                    op=mybir.AluOpType.add)
                t_rec = wpool.tile([P, HEADS], F32, tag="rec")
                nc.vector.reciprocal(t_rec[:], t_den[:])

                # weighted message sum U = sum_l e * h  (channel-major M)
                tM = wpool.tile([P, 2 * PT_max * HC], F32, tag="M")
                M3 = tM[:, : L * HC].rearrange("p (j l) -> p j l", l=L)
                e_b = E3h.unsqueeze(2).to_broadcast((P, HEADS, CH, L))
                nc.vector.tensor_tensor(
                    out=M3[:], in0=gT[:, 0:HC, :L], in1=e_b,
                    op=mybir.AluOpType.mult)
                tU = wpool.tile([P, HC], F32, tag="U")
                nc.vector.tensor_reduce(
                    out=tU[:], in_=M3, axis=mybir.AxisListType.X,
                    op=mybir.AluOpType.add)

                # out = relu(U / denom + bias)  (small V ops on gpsimd)
                tV = wpool.tile([P, HC], F32, tag="V")
                rec_b = t_rec[:].unsqueeze(2).to_broadcast((P, HEADS, CH))
                nc.vector.tensor_tensor(
                    out=tV[:].rearrange("p (h c) -> p h c", c=CH),
                    in0=tU[:].rearrange("p (h c) -> p h c", c=CH),
                    in1=rec_b, op=mybir.AluOpType.mult)
                nc.gpsimd.tensor_tensor(
                    out=tV[:], in0=tV[:], in1=t_bb[:], op=mybir.AluOpType.add)
                nc.scalar.activation(tV[:], tV[:], mybir.ActivationFunctionType.Relu)

                # rv = sum(V * W2) fused into accumulator (bf16 for the
                # pooling matmul: lhsT/rhs dtypes must match)
                tR = wpool.tile([P, HC], F32, tag="R")
                tRV = wpool.tile([P, 1], BF16, tag="RV")
                nc.vector.scalar_tensor_tensor(
                    out=tR[:], in0=tV[:], scalar=1.0, in1=t_w2[:],
                    op0=mybir.AluOpType.mult, op1=mybir.AluOpType.mult,
                    accum_out=tRV[:])

                # pool into batches: q += onehot(bid)^T @ rv  (host one-hot)
                ps_q = psB.tile([P, 1], F32, space="PSUM", tag="q")
                nc.tensor.matmul(
                    ps_q[:], lhsT=t_oh[:, t * P:(t + 1) * P], rhs=tRV[:],
                    start=True, stop=True)
                nc.vector.tensor_tensor(
                    out=t_qacc[:], in0=t_qacc[:], in1=ps_q[:],
                    op=mybir.AluOpType.add)

            nc.sync.dma_start(d_q[:], t_qacc[:])
    nc.finalize()
    _nc_cache[key] = nc
    return nc


def _pack_dbl(x, n_dbl, in_dim):
    """Pack [n_dbl*256, in_dim] node-major features into the K=128
    block-diagonal lhsT layout [2*in_dim, n_dbl*128] (bf16)."""
    a = x.reshape(n_dbl, 2, P, in_dim)
    return np.ascontiguousarray(
        a.transpose(1, 3, 0, 2).reshape(2 * in_dim, n_dbl * P)
    ).astype(ml_dtypes.bfloat16)


def kernel(**inputs):
    x_s = np.asarray(inputs["x_s"], np.float32)
    x_t = np.asarray(inputs["x_t"], np.float32)
    edge_index = np.asarray(inputs["edge_index"])
    x_s_batch = np.asarray(inputs["x_s_batch"]).astype(np.int64)
    W = np.asarray(inputs["W"], np.float32)
    att_src = np.asarray(inputs["att_src"], np.float32)
    att_dst = np.asarray(inputs["att_dst"], np.float32)
    bias = np.asarray(inputs["bias"], np.float32)
    fc1_w = np.asarray(inputs["fc1_w"], np.float32)
    fc1_b = np.asarray(inputs["fc1_b"], np.float32)
    fc3_w = np.asarray(inputs["fc3_w"], np.float32)
    fc3_b = np.asarray(inputs["fc3_b"], np.float32)

    n_nodes, in_dim = x_s.shape
    src = edge_index[0].astype(np.int64)
    dst = edge_index[1].astype(np.int64)

    # ---- host: edge bucketing by destination (layout prep only) ----
    deg = np.bincount(dst, minlength=n_nodes)
    order = np.argsort(-deg, kind="stable")      # nodes by degree desc
    nodes_per_core = (n_nodes + N_CORES - 1) // N_CORES
    n_dst_tiles = (nodes_per_core + P - 1) // P
    L_list = []
    for t in range(n_dst_tiles):
        r0 = t * P * N_CORES
        Lt = max(1, int(deg[order[min(r0, n_nodes - 1)]]))
        L_list.append(Lt + (Lt & 1))             # force even (2 slots/matmul)
    PT_list = [Lt // 2 for Lt in L_list]
    n_xt_dbl = (n_dst_tiles + 1) // 2

    # edges sorted by dst -> per-node contiguous src runs
    e_order = np.argsort(dst, kind="stable")
    src_sorted = src[e_order].astype(np.int64)
    starts = np.searchsorted(dst[e_order], np.arange(n_nodes))

    # fold weights (host weight prep)
    wa_t = np.einsum("khc,hc->kh", W.reshape(in_dim, HEADS, CH), att_dst).astype(np.float32)
    wa_s = np.einsum("khc,hc->kh", W.reshape(in_dim, HEADS, CH), att_src).astype(np.float32)
    wfold = np.zeros((in_dim, ROW), np.float32)
    wfold[:, :HC] = W
    wfold[:, HC:HC + HEADS] = wa_s
    wf2 = np.zeros((2 * in_dim, 2 * ROW), np.float32)
    wf2[:in_dim, :ROW] = wfold
    wf2[in_dim:, ROW:] = wfold
    # interleave output columns to (c, half) so PSUM->g copies land
    # channel-major: new col c*2+half = old col half*ROW+c
    wf2 = np.ascontiguousarray(
        wf2.reshape(2 * in_dim, 2, ROW).transpose(0, 2, 1).reshape(
            2 * in_dim, 2 * ROW)).astype(ml_dtypes.bfloat16)
    wat4 = np.zeros((in_dim, 4), np.float32)
    wat4[:, :HEADS] = wa_t
    wat2 = np.zeros((2 * in_dim, 8), np.float32)
    wat2[:in_dim, :4] = wat4
    wat2[in_dim:, 4:] = wat4
    wat2 = wat2.astype(ml_dtypes.bfloat16)
    w2 = (fc1_w @ fc3_w)[:, 0].astype(np.float32)      # [36]
    w2b = np.tile(w2[None, :], (P, 1))
    biasb = np.tile(bias[None, :], (P, 1))

    # pad-slot x vector: v_pad @ wa_s = PAD_A for every head
    v_pad = wa_s @ np.linalg.solve(
        wa_s.T @ wa_s, np.full((HEADS,), PAD_A, np.float64)).astype(np.float32)
    x_ext = np.vstack([x_s, v_pad[None, :]])           # row n_nodes = pad

    slot_ar = {}
    for Lt in set(L_list):
        slot_ar[Lt] = np.arange(Lt)[None, :]

    in_maps = []
    for c in range(N_CORES):
        node_ids = order[c::N_CORES]             # this core's dst nodes, deg-sorted
        ncnt = len(node_ids)
        pad_nodes = n_dst_tiles * P - ncnt
        nodes_pad = np.concatenate(
            [node_ids, np.zeros(pad_nodes, np.int64)]) if pad_nodes else node_ids
        valid_row = np.arange(n_dst_tiles * P) < ncnt

        oh = np.zeros((P, n_dst_tiles * P), np.float32)
        xe_blocks = []
        for t in range(n_dst_tiles):
            Lt = L_list[t]
            nt = nodes_pad[t * P:(t + 1) * P]
            vr = valid_row[t * P:(t + 1) * P]
            lens = np.where(vr, deg[nt], 0)
            mask = slot_ar[Lt] < lens[:, None]           # [P, Lt]
            idt = np.full((P, Lt), n_nodes, np.int64)
            gather_pos = (starts[nt][:, None] + slot_ar[Lt])[mask]
            idt[mask] = src_sorted[gather_pos]
            Et = x_ext[idt]                              # [P, Lt, in_dim]
            Et = Et.reshape(P, Lt // 2, 2, in_dim).transpose(2, 3, 1, 0)
            xe_blocks.append(np.ascontiguousarray(
                Et.reshape(2 * in_dim, (Lt // 2) * P)))
            rows = np.nonzero(vr)[0]
            oh[rows, t * P + x_s_batch[nt[rows]]] = 1.0
        xe = np.concatenate(xe_blocks, axis=1).astype(ml_dtypes.bfloat16)
        oh = oh.astype(ml_dtypes.bfloat16)

        valid = min(ncnt, n_dst_tiles * P)
        xt_pad = np.zeros((n_xt_dbl * 2 * P, in_dim), np.float32)
        xt_pad[:valid] = x_t[node_ids[:valid]]
        xt_p = _pack_dbl(xt_pad, n_xt_dbl, in_dim)
        in_maps.append({
            "xe": xe, "xt_p": xt_p, "oh": oh,
            "wf2": wf2, "wat2": wat2, "w2b": w2b, "biasb": biasb,
        })

    nc = _build_nc(in_dim, n_dst_tiles, n_xt_dbl, PT_list)
    res = run_bass_kernel_spmd(nc, in_maps, core_ids=list(range(N_CORES)))

    q = np.zeros((P,), np.float64)
    for c in range(N_CORES):
        q += res.results[c]["q_out"][:, 0]
    cnt = np.bincount(x_s_batch, minlength=P).astype(np.float64)
    out = q / np.maximum(cnt, 1.0)
    const = float(fc1_b @ fc3_w[:, 0] + fc3_b[0])
    return (out + const).astype(np.float32)
